# revision 12
# baseline (speedup 1.0000x reference)
"""Trainium2 Bass kernel for a 2-layer GRU decoder (B=128, T=512, H=512).

v3: both GRU layers fused into ONE hardware loop per core. Each loop
body covers 32 timesteps and interleaves: gru5 steps t..t+31, the
incremental xw6 = g5 @ W6 transform for the just-produced g5 block, and
gru6 steps t-32..t-1 consuming the xw6 block produced one body earlier.
The two recurrence chains are independent, so each layer's gate math
hides under the other layer's matmul stream — the step period becomes
engine-throughput-bound instead of latency-bound.

Gate math is all bf16 with the update rewritten as
h' = hh + zg*(h - hh); the z-gate add carries a bypass-ALU dependency
on q so the tile scheduler (whose sim does not model LDWEIGHTS) cannot
commit it ahead of the latency-critical q/hpre/tanh chain on the
in-order DVE stream.

gru6's first 32 steps run on zeroed xw6 blocks: with xw=0 the gates
come out zg=rg=0, hh=tanh(0)=0, so h6 stays exactly 0 until real data
arrives — no masking needed for pipeline warm-up.
"""

import numpy as np
import ml_dtypes

B, T, LAT, F2, H = 128, 512, 256, 64, 512
DIN = LAT + F2          # 320
G3 = 3 * H              # 1536
NCORES = 8
BL = B // NCORES        # 16 batch / core
NT = T * BL             # 8192 flat (t,b) rows per core
NCH = G3 // 128         # 12 output-dim chunks
KH = H // 128           # 4 hidden-dim chunks
KIN = 3                 # padded 384 input-dim chunks
NBULK = NT // 512       # 16 bulk column chunks of 512
TB = T // NBULK         # 32 timesteps per bulk chunk
NJ = NT // 128          # 64 dense output chunks
HU = 8                  # steps per half
SB = 2 * HU             # 16 steps per sub-body
BODY = 2 * SB           # 32 timesteps per loop body
TPAD = T + BODY + SB    # xw5 DRAM padded for the pipeline tail + prefetch
G6S = T + BODY          # g6 slot count (t6 stored at slot t6 + BODY)

bf16 = ml_dtypes.bfloat16
_CACHE = {}


def _build(bd_val, has_brh):
    import concourse.bass as bass
    import concourse.tile as tile
    import concourse.mybir as mybir
    from concourse import bacc
    from concourse.bass import ds

    f32 = mybir.dt.float32
    bf = mybir.dt.bfloat16
    AF = mybir.ActivationFunctionType
    OP = mybir.AluOpType
    ET = mybir.EngineType

    nc = bacc.Bacc(None, target_bir_lowering=False, debug=False)

    xt_d = nc.dram_tensor("xt_d", [KIN, 128, NT], bf, kind="ExternalInput")
    w5_d = nc.dram_tensor("w5_d", [KIN, 128, G3], bf, kind="ExternalInput")
    u5_d = nc.dram_tensor("u5_d", [KH, 128, G3], bf, kind="ExternalInput")
    w6_d = nc.dram_tensor("w6_d", [KH, 128, G3], bf, kind="ExternalInput")
    u6_d = nc.dram_tensor("u6_d", [KH, 128, G3], bf, kind="ExternalInput")
    b5_d = nc.dram_tensor("b5_d", [128, NCH], f32, kind="ExternalInput")
    b6_d = nc.dram_tensor("b6_d", [128, NCH], f32, kind="ExternalInput")
    br5_d = nc.dram_tensor("br5_d", [128, KH], f32, kind="ExternalInput")
    br6_d = nc.dram_tensor("br6_d", [128, KH], f32, kind="ExternalInput")
    wd_d = nc.dram_tensor("wd_d", [128, KH], bf, kind="ExternalInput")
    dm_d = nc.dram_tensor("dm_d", [128, NJ], f32, kind="ExternalInput")
    out_d = nc.dram_tensor("out_d", [128, NJ], f32, kind="ExternalOutput")

    with tile.TileContext(nc) as tc:
        import contextlib
        stack = contextlib.ExitStack()
        with stack:
            drp = stack.enter_context(tc.tile_pool(name="dram", bufs=1, space="DRAM"))
            xw5_t = drp.tile([128, TPAD, NCH, BL], bf)

            pp = stack.enter_context(tc.tile_pool(name="persist", bufs=1))
            b5_sb = pp.tile([128, NCH], f32, tag="b5")
            b6_sb = pp.tile([128, NCH], f32, tag="b6")
            nc.sync.dma_start(b5_sb[:], b5_d.ap()[:])
            nc.sync.dma_start(b6_sb[:], b6_d.ap()[:])
            scr = pp.tile([128, 1], f32, tag="scr")
            nc.vector.tensor_tensor(
                scr[:], b5_sb[:, 0:1], b6_sb[:, 0:1],
                op=mybir.AluOpType.add,
            )
            # preload the tanh activation table once, outside all loops
            nc.scalar.activation(scr[:], scr[:], AF.Tanh)
            if has_brh:
                brh5_sb = pp.tile([128, KH], f32, tag="brh5")
                brh6_sb = pp.tile([128, KH], f32, tag="brh6")
                nc.sync.dma_start(brh5_sb[:], br5_d.ap()[:])
                nc.sync.dma_start(brh6_sb[:], br6_d.ap()[:])

            # persistent loop state
            xw_a = pp.tile([128, HU, NCH, BL], bf, tag="xwa")
            xw_b = pp.tile([128, HU, NCH, BL], bf, tag="xwb")
            # g5 history: 1 block per sub-body (16 steps)
            h5 = [pp.tile([128, KH, SB, BL], bf, tag=f"h5_{i}", name=f"h5_{i}")
                  for i in range(2)]
            # g6 history: 1 block per sub-body
            h6 = [pp.tile([128, KH, SB, BL], bf, tag=f"h6_{i}", name=f"h6_{i}")
                  for i in range(2)]
            # xw6 blocks: written by sub-body s, consumed by sub-body s+1
            xw6 = [pp.tile([128, SB, NCH, BL], bf, tag=f"xw6_{i}", name=f"xw6_{i}")
                   for i in range(2)]
            # full g6 kept in SBUF for the dense phase (slot = t6 + BODY)
            g6_sb = pp.tile([128, KH, G6S, BL], bf, tag="g6")
            zero64 = pp.tile([128, 64], bf, tag="zero64")
            nc.vector.memset(zero64[:], 0.0)

            # ---------- Phase 1: xw5 = (X @ W5) + bias ----------
            with (
                tc.tile_pool(name="p1x", bufs=1) as xp,
                tc.tile_pool(name="blkps", bufs=4, space="PSUM") as psp1,
                tc.tile_pool(name="blko", bufs=2) as op1,
            ):
                x_sb = xp.tile([128, KIN, NT], bf)
                nc.sync.dma_start(
                    x_sb[:], xt_d.ap().rearrange("k p n -> p k n")
                )
                w_sb = xp.tile([128, KIN, G3], bf, tag="w5sb")
                nc.sync.dma_start(
                    w_sb[:], w5_d.ap().rearrange("k p g -> p k g")
                )
                for n in range(NBULK):
                    ob = op1.tile([128, TB, NCH, BL], bf, tag="ob")
                    for c in range(NCH):
                        ps = psp1.tile([128, 512], f32, tag="ps")
                        for k in range(KIN):
                            nc.tensor.matmul(
                                ps[:],
                                w_sb[:, k, c * 128:(c + 1) * 128],
                                x_sb[:, k, n * 512:(n + 1) * 512],
                                start=(k == 0),
                                stop=(k == KIN - 1),
                            )
                        nc.vector.tensor_scalar(
                            ob[:, :, c, :],
                            ps[:].rearrange("p (t b) -> p t b", b=BL),
                            b5_sb[:, c:c + 1],
                            None,
                            op0=OP.add,
                        )
                    nc.sync.dma_start(
                        xw5_t[:, n * TB:(n + 1) * TB, :, :], ob[:]
                    )

            # ---------- fused dual-layer GRU + in-loop xw6 ----------
            with (
                tc.tile_pool(name="guw", bufs=1) as up,
                tc.tile_pool(name="gps", bufs=2, space="PSUM") as psp,
                tc.tile_pool(name="bps", bufs=2, space="PSUM") as bpsp,
                tc.tile_pool(name="gwk", bufs=3) as wk,
            ):
                u5_sb = up.tile([128, KH, G3], bf, tag="u5sb")
                nc.sync.dma_start(
                    u5_sb[:], u5_d.ap().rearrange("k p g -> p k g")
                )
                u6_sb = up.tile([128, KH, G3], bf, tag="u6sb")
                nc.sync.dma_start(
                    u6_sb[:], u6_d.ap().rearrange("k p g -> p k g")
                )
                w6_sb = up.tile([128, KH, G3], bf, tag="w6sb")
                nc.sync.dma_start(
                    w6_sb[:], w6_d.ap().rearrange("k p g -> p k g")
                )
                # zero-init pipeline state
                for t_ in h5:
                    nc.vector.memset(t_[:], 0.0)
                for t_ in h6:
                    nc.vector.memset(t_[:], 0.0)
                for t_ in xw6:
                    nc.vector.memset(t_[:], 0.0)
                # prime first xw5 half-block
                nc.sync.dma_start(xw_a[:], xw5_t[:, 0:HU, :, :])

                def step5(xwblk, xwi, hist, hist_prev, uu):
                    """One gru5 step. hist/hist_prev: [128,KH,SB,BL]."""
                    h_prev = (hist_prev[:, :, SB - 1, :] if uu == 0
                              else hist[:, :, uu - 1, :])
                    _gru_step(u5_sb, xwblk, xwi, h_prev,
                              hist[:, :, uu, :],
                              brh5_sb if has_brh else None)

                def step6(xwblk, hist, hist_prev, jj):
                    """One gru6 step. hist/hist_prev: [128,KH,SB,BL]."""
                    h_prev = (hist_prev[:, :, SB - 1, :] if jj == 0
                              else hist[:, :, jj - 1, :])
                    _gru_step(u6_sb, xwblk, jj, h_prev,
                              hist[:, :, jj, :],
                              brh6_sb if has_brh else None)

                def _gru_step(u_sb, xwblk, uu, h_prev, h_out, brh_sb):
                    ps_r = psp.tile([128, 4 * BL], f32, tag="psr")
                    ps_ih = psp.tile([128, 4 * BL], f32, tag="psih")
                    ps_z = psp.tile([128, 4 * BL], f32, tag="psz")

                    def mm_gate(tgt, c0, pre):
                        for cc in range(4):
                            c = c0 + cc
                            col = cc * BL
                            for k in range(KH):
                                nc.tensor.matmul(
                                    tgt[:, col:col + BL],
                                    u_sb[:, k, c * 128:(c + 1) * 128],
                                    h_prev[:, k, :],
                                    start=(not pre and cc == 0 and k == 0),
                                    stop=(cc == 3 and k == KH - 1),
                                    skip_group_check=True,
                                )

                    mm_gate(ps_r, 4, False)
                    mm_gate(ps_ih, 8, False)
                    mm_gate(ps_z, 0, False)

                    sr = wk.tile([128, 64], f32, tag="sr")
                    nc.vector.tensor_tensor(
                        sr[:],
                        xwblk[:, uu, 4:8, :].rearrange("p c b -> p (c b)"),
                        ps_r[:], op=OP.add,
                    )
                    rg = wk.tile([128, 64], bf, tag="rg")
                    nc.vector.tensor_scalar(
                        rg[:], sr[:], 0.0, 1.0, op0=OP.max, op1=OP.min
                    )
                    if brh_sb is not None:
                        nc.vector.tensor_tensor(
                            ps_ih[:], ps_ih[:],
                            brh_sb[:].rearrange("p k -> p k 1")
                            .broadcast(2, BL)
                            .rearrange("p k b -> p (k b)"),
                            op=OP.add,
                        )
                    q = wk.tile([128, 64], bf, tag="q")
                    nc.vector.tensor_tensor(
                        q[:], rg[:], ps_ih[:], op=OP.mult
                    )
                    hpre = wk.tile([128, 64], bf, tag="hpre")
                    nc.vector.tensor_tensor(
                        hpre[:], q[:],
                        xwblk[:, uu, 8:12, :].rearrange("p c b -> p (c b)"),
                        op=OP.add,
                    )
                    hh = wk.tile([128, 64], bf, tag="hh")
                    nc.scalar.activation(hh[:], hpre[:], AF.Tanh)
                    # zg rides a bypass dep on q so the scheduler can't
                    # commit it ahead of the latency-critical ih chain
                    zz = wk.tile([128, 64], f32, tag="zz")
                    nc.vector.scalar_tensor_tensor(
                        zz[:],
                        xwblk[:, uu, 0:4, :].rearrange("p c b -> p (c b)"),
                        q[:, 0:1],
                        ps_z[:], op0=OP.bypass, op1=OP.add,
                    )
                    zg = wk.tile([128, 64], bf, tag="zg")
                    nc.vector.tensor_scalar(
                        zg[:], zz[:], 0.0, 1.0, op0=OP.max, op1=OP.min
                    )
                    # h' = hh + zg * (h_prev - hh), all bf16
                    dd = wk.tile([128, KH, BL], bf, tag="dd")
                    nc.vector.tensor_tensor(
                        dd[:], h_prev,
                        hh[:].rearrange("p (k b) -> p k b", b=BL),
                        op=OP.subtract,
                    )
                    mm_ = wk.tile([128, KH, BL], bf, tag="mm")
                    nc.vector.tensor_tensor(
                        mm_[:],
                        zg[:].rearrange("p (k b) -> p k b", b=BL),
                        dd[:], op=OP.mult,
                    )
                    nc.vector.tensor_tensor(
                        h_out,
                        hh[:].rearrange("p (k b) -> p k b", b=BL),
                        mm_[:], op=OP.add,
                    )

                def bulk6(src, dst, cs):
                    """xw6 for 16 steps: dst[:, :, c, :] for c in cs."""
                    for c in cs:
                        ps = bpsp.tile([128, 2, HU * BL], f32, tag="bps")
                        for k in range(KH):
                            nc.tensor.matmul(
                                ps[:].rearrange("p a b -> p (a b)"),
                                w6_sb[:, k, c * 128:(c + 1) * 128],
                                src[:, k, :, :]
                                .rearrange("p u b -> p (u b)"),
                                start=(k == 0),
                                stop=(k == KH - 1),
                                skip_group_check=True,
                            )
                        nc.scalar.activation(
                            dst[:, :, c, :],
                            ps[:].rearrange("p h (u b) -> p (h u) b", b=BL),
                            AF.Identity, bias=b6_sb[:, c:c + 1],
                        )

                def sub_body(iv, s):
                    """Sub-body s (0/1) of the body starting at iv."""
                    # gru5 block for this sub-body
                    h5c, h5p = h5[s], h5[1 - s]
                    # gru6 consumes xw6[1-s], produces hist into h6[s]
                    xw6_c = xw6[1 - s]
                    xw6_p = xw6[s]
                    h6c, h6p = h6[s], h6[1 - s]

                    off = iv if s == 0 else iv + SB

                    # prefetch xw5 for the second half, then first-half work
                    nc.sync.dma_start(
                        xw_b[:], xw5_t[:, ds(off + HU, HU), :, :]
                    )
                    for j in range(HU):
                        step5(xw_a, j, h5c, h5p, j)
                        bulk6(h5p, xw6_p, [j] if j < 6 else [])
                        step6(xw6_c, h6c, h6p, j)
                    nc.sync.dma_start(
                        xw_a[:], xw5_t[:, ds(off + 2 * HU, HU), :, :]
                    )
                    for j in range(HU):
                        step5(xw_b, j, h5c, h5p, HU + j)
                        bulk6(h5p, xw6_p, [6 + j] if j < 6 else [])
                        step6(xw6_c, h6c, h6p, HU + j)
                    # stash gru6 output (t6 = off - BODY, stored at slot off)
                    nc.sync.dma_start(
                        g6_sb[:, :, ds(off, SB), :], h6c[:]
                    )

                with tc.For_i(
                    0, T + BODY, BODY,
                    hint_engines=(ET.PE, ET.DVE, ET.Activation),
                    staggered_reset=True,
                ) as iv:
                    sub_body(iv, 0)
                    sub_body(iv, 1)

            # ---------- dense: dec = tanh(g6 @ Wd + bd) * dec_mask ----------
            with (
                tc.tile_pool(name="p5ps", bufs=2, space="PSUM") as psp5,
                tc.tile_pool(name="p5o", bufs=1) as op5,
            ):
                wd_sb = op5.tile([128, KH], bf, tag="wd")
                nc.sync.dma_start(wd_sb[:], wd_d.ap()[:])
                dm_sb = op5.tile([128, NJ], f32, tag="dm")
                nc.sync.dma_start(dm_sb[:], dm_d.ap()[:])
                ps_d = psp5.tile([128, NJ], f32, tag="psd")
                TJ = 128 // BL  # 8 timesteps per dense chunk
                for j in range(NJ):
                    for k in range(KH):
                        nc.tensor.matmul(
                            ps_d[:, j:j + 1],
                            g6_sb[:, k, BODY + j * TJ:BODY + (j + 1) * TJ, :]
                            .rearrange("p t b -> p (t b)"),
                            wd_sb[:, k:k + 1],
                            start=(j == 0 and k == 0),
                            stop=(j == NJ - 1 and k == KH - 1),
                            skip_group_check=True,
                        )
                dec = op5.tile([128, NJ], f32, tag="dec")
                nc.scalar.activation(
                    dec[:], ps_d[:], AF.Tanh, bias=float(bd_val), scale=1.0
                )
                nc.vector.tensor_tensor(dec[:], dec[:], dm_sb[:], op=OP.mult)
                nc.sync.dma_start(out_d.ap()[:], dec[:])

    nc.compile()
    return nc


def _prep(inputs):
    """Host-side: shard on batch, permute/pad/cast into device layouts."""
    z = np.asarray(inputs["z"], np.float32)
    x2 = np.asarray(inputs["train_input_two"], np.float32)
    masks = np.asarray(inputs["masks"], np.float32)
    dmasks = np.asarray(inputs["dec_masks"], np.float32)
    W5 = np.asarray(inputs["W5"], np.float32)
    U5 = np.asarray(inputs["U5"], np.float32)
    bi5 = np.asarray(inputs["bi5"], np.float32)
    br5 = np.asarray(inputs["br5"], np.float32)
    W6 = np.asarray(inputs["W6"], np.float32)
    U6 = np.asarray(inputs["U6"], np.float32)
    bi6 = np.asarray(inputs["bi6"], np.float32)
    br6 = np.asarray(inputs["br6"], np.float32)
    Wd = np.asarray(inputs["Wd"], np.float32)
    bd = np.asarray(inputs["bd"], np.float32)

    def scale_w(W):  # scale z,r columns by 0.2 (hard-sigmoid prescale)
        Ws = W.copy()
        Ws[:, : 2 * H] *= 0.2
        return Ws

    def pack_w(W, kdim):  # [D,G3] -> [kdim,128,G3] bf16 (zero-padded)
        D = W.shape[0]
        Wp = np.zeros((kdim * 128, G3), np.float32)
        Wp[:D] = W
        return np.ascontiguousarray(
            Wp.reshape(kdim, 128, G3).astype(bf16)
        )

    def pack_bias(bi, br):  # xw-path bias, [128, NCH] (partition, chunk)
        bt = np.empty(G3, np.float32)
        bt[: 2 * H] = 0.2 * (bi[: 2 * H] + br[: 2 * H]) + 0.5
        bt[2 * H:] = bi[2 * H:]
        return np.ascontiguousarray(bt.reshape(NCH, 128).T)

    w5p = pack_w(scale_w(W5), KIN)
    u5p = pack_w(scale_w(U5), KH)
    w6p = pack_w(scale_w(W6), KH)
    u6p = pack_w(scale_w(U6), KH)
    b5p = pack_bias(bi5, br5)
    b6p = pack_bias(bi6, br6)
    brh5 = np.ascontiguousarray(br5[2 * H:].reshape(KH, 128).T)
    brh6 = np.ascontiguousarray(br6[2 * H:].reshape(KH, 128).T)
    has_brh = bool(np.any(brh5) or np.any(brh6))
    wdp = np.ascontiguousarray(Wd[:, 0].reshape(KH, 128).T.astype(bf16))

    # masked concat input, transposed: XT [384, T*BL] per core
    rep = np.broadcast_to(z[:, None, :], (B, T, LAT))
    X = np.concatenate([rep, x2], axis=-1) * masks  # [B,T,320]

    in_maps = []
    for cidx in range(NCORES):
        sl = slice(cidx * BL, (cidx + 1) * BL)
        Xc = X[sl]                                    # [BL,T,320]
        XT = np.zeros((KIN * 128, NT), np.float32)
        XT[:DIN] = Xc.transpose(2, 1, 0).reshape(DIN, NT)  # (d,t,b)
        dmc = dmasks[sl, :, 0].T.reshape(NT)          # flat t*BL+b
        in_maps.append({
            "xt_d": np.ascontiguousarray(
                XT.reshape(KIN, 128, NT).astype(bf16)),
            "w5_d": w5p, "u5_d": u5p, "w6_d": w6p, "u6_d": u6p,
            "b5_d": b5p, "b6_d": b6p,
            "br5_d": brh5, "br6_d": brh6,
            "wd_d": wdp,
            "dm_d": np.ascontiguousarray(dmc.reshape(NJ, 128).T),
        })
    return in_maps, has_brh, float(bd.reshape(-1)[0])


def kernel(**inputs):
    from concourse.bass_utils import run_bass_kernel_spmd

    in_maps, has_brh, bd_val = _prep(inputs)
    key = (has_brh, bd_val)
    if key not in _CACHE:
        _CACHE[key] = _build(bd_val, has_brh)
    nc = _CACHE[key]
    res = run_bass_kernel_spmd(nc, in_maps, core_ids=list(range(NCORES)))
    out = np.empty((B, T, 1), np.float32)
    for cidx in range(NCORES):
        o = res.results[cidx]["out_d"]                # [128, NJ]
        flat = o.T.reshape(NT)                        # flat = t*BL + b
        out[cidx * BL:(cidx + 1) * BL, :, 0] = flat.reshape(T, BL).T
    return out
